# revision 21
# baseline (speedup 1.0000x reference)
"""ChebyKAN layer kernel for 8x Trainium2 NeuronCores.

Computes y[b,o] = sum_{i,d} T_d(tanh(x[b,i])) * C[i,o,d], d = 0..8,
with T_d the Chebyshev polynomials, via:
  - batch sharded 8 ways (1024 rows/core)
  - device computes T_1..T_8 with Chebyshev product identities
    (fp32 DVE/ACT), casts basis to bf16
  - d=0 term (T_0 == 1) folded into a host-precomputed bias[o]
  - big contraction as bf16 matmuls accumulating fp32 in PSUM:
    K = (i,d) of size 8192 in 64 chunks of 128
  - x is transposed on host so the basis is produced directly in
    [K, batch] (lhsT) layout; no on-device transpose needed.

Self-contained: hardcodes all shapes for inputs
  x: [8192, 1024] f32, cheby_coeffs: [1024, 1024, 9] f32.
"""

import numpy as np
import ml_dtypes

import concourse.bass as bass
import concourse.mybir as mybir
import concourse.tile as tile
from concourse import bacc
from concourse.bass_utils import run_bass_kernel_spmd

P = 128
B_TOTAL = 8192
I_DIM = 1024
O_DIM = 1024
DEG = 8              # degrees 1..8 on device (d=0 folded into bias)
N_CORES = 8
B_LOCAL = B_TOTAL // N_CORES     # 1024
IC = I_DIM // P                  # 8 input chunks
NK = IC * DEG                    # 64 K-chunks of 128
OH = 2                           # output halves (PSUM capacity: 8 banks)
ON = O_DIM // OH                 # 512

_nc = None
last_results = None  # BassKernelResults of the most recent run (for profiling)


def _build_nc():
    nc = bacc.Bacc()
    f32 = mybir.dt.float32
    bf16 = mybir.dt.bfloat16
    AF = mybir.ActivationFunctionType
    ALU = mybir.AluOpType

    xt_d = nc.dram_tensor("xt", [I_DIM, B_LOCAL], f32, kind="ExternalInput")
    w_d = nc.dram_tensor("w", [OH, NK, P, ON], bf16, kind="ExternalInput")
    bias_d = nc.dram_tensor("bias", [P, O_DIM], f32, kind="ExternalInput")
    y_d = nc.dram_tensor("y", [B_LOCAL, O_DIM], f32, kind="ExternalOutput")

    with tile.TileContext(nc) as tc:
        with (
            tc.tile_pool(name="const", bufs=1) as cpool,
            tc.tile_pool(name="xin", bufs=2) as xpool,
            tc.tile_pool(name="fwork", bufs=2) as fpool,
            tc.tile_pool(name="basis", bufs=1) as bpool,
            tc.tile_pool(name="wstream", bufs=8) as wpool,
            tc.tile_pool(name="outbuf", bufs=4) as opool,
            tc.tile_pool(name="acc", bufs=1, space="PSUM") as ppool,
        ):
            # ---- PE warm-up ----
            # HAM un-throttles the PE clock (1.2 -> 2.4 GHz) only after
            # ~3.4us of sustained matmul activity. Burn that window on dummy
            # matmuls into psum bank 0 while the first xt/wt DMAs are in
            # flight; the real k=0 matmul re-starts the bank (start=True).
            warm = cpool.tile([P, ON], bf16, name="warm")
            nc.vector.memset(warm, 1.0)
            warm_ps = ppool.tile([P, ON], f32, tag="ps0", name="warm_ps")
            for wi in range(14):
                nc.tensor.matmul(warm_ps, warm[:, 0:P], warm,
                                 start=(wi == 0), stop=(wi == 13))

            # ---- basis production: T_1..T_8 per 128-row chunk of i ----
            basis = {}

            def emit_cast(ic, d, src, eng):
                bt = bpool.tile([P, B_LOCAL], bf16, tag=f"b_{ic}_{d}",
                                name=f"b_{ic}_{d}")
                if eng == "dve":
                    nc.vector.tensor_copy(bt, src)
                elif eng == "gpsimd":
                    nc.gpsimd.tensor_copy(bt, src)
                else:
                    nc.scalar.copy(bt, src)
                basis[(ic, d)] = bt

            for ic in range(IC):
                # xt on the HWDGE (sync) queue: issues in parallel with the
                # gpsimd wt stream and has lower first-byte latency, so the
                # first tanh can start ~5us sooner.
                xt_t = xpool.tile([P, B_LOCAL], f32, tag="xt", name=f"xt_{ic}")
                if ic == 0:
                    # split the first load so tanh on the first half can
                    # start as soon as half the data lands
                    h = B_LOCAL // 2
                    nc.sync.dma_start(out=xt_t[:, 0:h], in_=xt_d[0:P, 0:h])
                    nc.sync.dma_start(out=xt_t[:, h:], in_=xt_d[0:P, h:])
                else:
                    nc.sync.dma_start(out=xt_t,
                                      in_=xt_d[ic * P:(ic + 1) * P, :])

                # T1 = tanh(x) (no clip: the recurrence is stable for |t|<=1
                # and T_d(+-1) is finite; deviation from the reference's
                # clip at 0.999 is ~1e-6 on y)
                t = fpool.tile([P, B_LOCAL], f32, tag="T1", name=f"t_{ic}")
                if ic == 0:
                    # split the very first tanh+cast in half so the first
                    # matmul (which only reads b_0_1[:, 0:128]) issues as
                    # soon as the first half is cast.
                    bt0 = bpool.tile([P, B_LOCAL], bf16, tag="b_0_1",
                                     name="b_0_1")
                    for h in range(2):
                        sl = slice(h * (B_LOCAL // 2), (h + 1) * (B_LOCAL // 2))
                        nc.scalar.activation(t[:, sl], xt_t[:, sl], AF.Tanh)
                        nc.vector.tensor_copy(bt0[:, sl], t[:, sl])
                    basis[(ic, 1)] = bt0
                else:
                    nc.scalar.activation(t, xt_t, AF.Tanh)
                    # DVE cast: shortens the tanh -> first-matmul chain
                    emit_cast(ic, 1, t, "dve")
                b1 = basis[(ic, 1)]

                # T2 = 2 t^2 - 1
                s2 = fpool.tile([P, B_LOCAL], f32, tag="sq", name=f"s2_{ic}")
                nc.scalar.square(s2, t)
                T2 = fpool.tile([P, B_LOCAL], f32, tag="T2", name=f"T2_{ic}", bufs=1)
                nc.vector.tensor_scalar(T2, s2, 2.0, -1.0, ALU.mult, ALU.add)
                emit_cast(ic, 2, T2, "act")
                b2 = basis[(ic, 2)]

                # T3 = 2 t T2 - t = t * (2 T2 - 1)
                V3 = fpool.tile([P, B_LOCAL], f32, tag="u", name=f"V3_{ic}")
                nc.vector.tensor_scalar(V3, T2, 2.0, -1.0, ALU.mult, ALU.add)
                T3 = fpool.tile([P, B_LOCAL], f32, tag="T3", name=f"T3_{ic}", bufs=1)
                nc.vector.tensor_mul(T3, t, V3)
                emit_cast(ic, 3, T3, "act")
                b3 = basis[(ic, 3)]

                # T4 = 2 T2^2 - 1
                s4 = fpool.tile([P, B_LOCAL], f32, tag="sq", name=f"s4_{ic}")
                nc.scalar.square(s4, T2)
                T4 = fpool.tile([P, B_LOCAL], f32, tag="T4", name=f"T4_{ic}", bufs=1)
                nc.vector.tensor_scalar(T4, s4, 2.0, -1.0, ALU.mult, ALU.add)
                emit_cast(ic, 4, T4, "act")
                b4 = basis[(ic, 4)]

                # Degrees 5..8 are leaves (no downstream consumer), so they
                # can be produced in cheaper precision/modes:
                #   T5 = 2 T2 T3 - T1, T7 = 2 T3 T4 - T1  from bf16 operands
                #   (bf16 DVE ops run in 2x mode; error stays ~bf16-level)
                #   T6 = 2 T3^2 - 1, T8 = 2 T4^2 - 1  as single tensor_scalar
                #   with direct bf16 output (fp32 squares from ACT)
                u5 = fpool.tile([P, B_LOCAL], bf16, tag="ub", name=f"u5_{ic}")
                nc.vector.tensor_mul(u5, b2, b3)
                b5t = bpool.tile([P, B_LOCAL], bf16, tag=f"b_{ic}_5",
                                 name=f"b_{ic}_5")
                nc.vector.scalar_tensor_tensor(b5t, u5, 2.0, b1,
                                               ALU.mult, ALU.subtract)
                basis[(ic, 5)] = b5t

                s6 = fpool.tile([P, B_LOCAL], f32, tag="sq", name=f"s6_{ic}")
                nc.scalar.square(s6, T3)
                b6t = bpool.tile([P, B_LOCAL], bf16, tag=f"b_{ic}_6",
                                 name=f"b_{ic}_6")
                nc.vector.tensor_scalar(b6t, s6, 2.0, -1.0, ALU.mult, ALU.add)
                basis[(ic, 6)] = b6t

                u7 = fpool.tile([P, B_LOCAL], bf16, tag="ub", name=f"u7_{ic}")
                nc.vector.tensor_mul(u7, b3, b4)
                b7t = bpool.tile([P, B_LOCAL], bf16, tag=f"b_{ic}_7",
                                 name=f"b_{ic}_7")
                nc.vector.scalar_tensor_tensor(b7t, u7, 2.0, b1,
                                               ALU.mult, ALU.subtract)
                basis[(ic, 7)] = b7t

                s8 = fpool.tile([P, B_LOCAL], f32, tag="sq", name=f"s8_{ic}")
                nc.scalar.square(s8, T4)
                b8t = bpool.tile([P, B_LOCAL], bf16, tag=f"b_{ic}_8",
                                 name=f"b_{ic}_8")
                nc.vector.tensor_scalar(b8t, s8, 2.0, -1.0, ALU.mult, ALU.add)
                basis[(ic, 8)] = b8t

            # bias is only consumed at the end of each o-half pass; load it
            # late so it doesn't delay the xt/wt streams.
            bias_t = cpool.tile([P, O_DIM], f32, name="bias_t")
            nc.sync.dma_start(out=bias_t, in_=bias_d[:, :])

            # ---- contraction: two o-half passes over all 64 K-chunks ----
            psums = [ppool.tile([P, ON], f32, tag=f"ps{b}", name=f"ps{b}")
                     for b in range(B_LOCAL // P)]
            # pass 0: o-half 0, all 8 batch banks (overlaps basis production)
            # pass 1a/1b: o-half 1 split in two bank halves, so the first
            # half's bias-adds + stores overlap the second half's matmuls
            # and the final tail only drains 4 banks.
            passes = [(0, 0, 8), (1, 0, 4), (1, 4, 8)]
            for pi, (oh, blo, bhi) in enumerate(passes):
                for k in range(NK):
                    ic, dm1 = divmod(k, DEG)
                    wt = wpool.tile([P, ON], bf16, tag="wt",
                                    name=f"wt_{pi}_{k}")
                    nc.gpsimd.dma_start(out=wt, in_=w_d[oh, k])
                    bt = basis[(ic, dm1 + 1)]
                    for b in range(blo, bhi):
                        nc.tensor.matmul(
                            psums[b],
                            bt[:, b * P:(b + 1) * P],
                            wt,
                            start=(k == 0),
                            stop=(k == NK - 1),
                        )
                for b in range(blo, bhi):
                    ot = opool.tile([P, ON], f32, tag="ot", name=f"ot_{pi}_{b}")
                    bias_sl = bias_t[:, oh * ON:(oh + 1) * ON]
                    if pi == 0 and b < 4:
                        # banks 0-3 gate pass 1a: drain them via ACT copy so
                        # the start=True matmuls aren't stuck behind the
                        # serial DVE bias-add chain; add bias in place later
                        # (overlaps the next pass).
                        nc.scalar.copy(ot, psums[b])
                        nc.vector.tensor_add(ot, ot, bias_sl)
                    else:
                        nc.vector.tensor_add(ot, psums[b], bias_sl)
                    nc.sync.dma_start(
                        out=y_d[b * P:(b + 1) * P, oh * ON:(oh + 1) * ON],
                        in_=ot)
    nc.compile()  # bacc legalization: splits multi-sem waits (TRN2 allows 1)
    return nc


def _get_nc():
    global _nc
    if _nc is None:
        _nc = _build_nc()
    return _nc


def _prep_inputs(x, cheby_coeffs):
    x = np.asarray(x, dtype=np.float32)
    C = np.asarray(cheby_coeffs, dtype=np.float32)
    bf16 = ml_dtypes.bfloat16

    # W[oh, k=(ic,d), p, on] = C[ic*128+p, oh*512+on, d+1]
    Wd = C[:, :, 1:]                                   # [I, O, 8]
    Wd = Wd.reshape(IC, P, OH, ON, DEG)                # [ic, p, oh, on, d]
    Wd = np.transpose(Wd, (2, 0, 4, 1, 3))             # [oh, ic, d, p, on]
    Wd = np.ascontiguousarray(Wd.reshape(OH, NK, P, ON)).astype(bf16)

    bias = C[:, :, 0].sum(axis=0, dtype=np.float64).astype(np.float32)
    bias_rep = np.ascontiguousarray(np.broadcast_to(bias, (P, O_DIM)))

    in_maps = []
    for c in range(N_CORES):
        xt = np.ascontiguousarray(x[c * B_LOCAL:(c + 1) * B_LOCAL, :].T)
        in_maps.append({"xt": xt, "w": Wd, "bias": bias_rep})
    return in_maps


def kernel(x, cheby_coeffs):
    global last_results
    nc = _get_nc()
    in_maps = _prep_inputs(x, cheby_coeffs)
    last_results = run_bass_kernel_spmd(nc, in_maps,
                                        core_ids=list(range(N_CORES)))
    y = np.concatenate([r["y"] for r in last_results.results], axis=0)
    return y


# revision 23
# speedup vs baseline: 1.0127x; 1.0127x over previous
"""ChebyKAN layer kernel for 8x Trainium2 NeuronCores.

Computes y[b,o] = sum_{i,d} T_d(tanh(x[b,i])) * C[i,o,d], d = 0..8,
with T_d the Chebyshev polynomials, via:
  - batch sharded 8 ways (1024 rows/core)
  - device computes T_1..T_8 with Chebyshev product identities
    (fp32 DVE/ACT), casts basis to bf16
  - d=0 term (T_0 == 1) folded into a host-precomputed bias[o]
  - big contraction as bf16 matmuls accumulating fp32 in PSUM:
    K = (i,d) of size 8192 in 64 chunks of 128
  - x is transposed on host so the basis is produced directly in
    [K, batch] (lhsT) layout; no on-device transpose needed.

Self-contained: hardcodes all shapes for inputs
  x: [8192, 1024] f32, cheby_coeffs: [1024, 1024, 9] f32.
"""

import numpy as np
import ml_dtypes

import concourse.bass as bass
import concourse.mybir as mybir
import concourse.tile as tile
from concourse import bacc
from concourse.bass_utils import run_bass_kernel_spmd

P = 128
B_TOTAL = 8192
I_DIM = 1024
O_DIM = 1024
DEG = 8              # degrees 1..8 on device (d=0 folded into bias)
N_CORES = 8
B_LOCAL = B_TOTAL // N_CORES     # 1024
IC = I_DIM // P                  # 8 input chunks
NK = IC * DEG                    # 64 K-chunks of 128
OH = 2                           # output halves (PSUM capacity: 8 banks)
ON = O_DIM // OH                 # 512

_nc = None
last_results = None  # BassKernelResults of the most recent run (for profiling)


def _build_nc():
    nc = bacc.Bacc()
    f32 = mybir.dt.float32
    bf16 = mybir.dt.bfloat16
    AF = mybir.ActivationFunctionType
    ALU = mybir.AluOpType

    xt_d = nc.dram_tensor("xt", [I_DIM, B_LOCAL], f32, kind="ExternalInput")
    w_d = nc.dram_tensor("w", [OH, NK, P, ON], bf16, kind="ExternalInput")
    bias_d = nc.dram_tensor("bias", [P, O_DIM], f32, kind="ExternalInput")
    y_d = nc.dram_tensor("y", [B_LOCAL, O_DIM], f32, kind="ExternalOutput")

    with tile.TileContext(nc) as tc:
        with (
            tc.tile_pool(name="const", bufs=1) as cpool,
            tc.tile_pool(name="xin", bufs=2) as xpool,
            tc.tile_pool(name="fwork", bufs=2) as fpool,
            tc.tile_pool(name="basis", bufs=1) as bpool,
            tc.tile_pool(name="wstream", bufs=8) as wpool,
            tc.tile_pool(name="outbuf", bufs=4) as opool,
            tc.tile_pool(name="acc", bufs=1, space="PSUM") as ppool,
        ):
            # ---- PE warm-up ----
            # HAM un-throttles the PE clock (1.2 -> 2.4 GHz) only after
            # ~3.4us of sustained matmul activity. Burn that window on dummy
            # matmuls into psum bank 0 while the first xt/wt DMAs are in
            # flight; the real k=0 matmul re-starts the bank (start=True).
            warm = cpool.tile([P, ON], bf16, name="warm")
            nc.vector.memset(warm, 1.0)
            warm_ps = ppool.tile([P, ON], f32, tag="ps0", name="warm_ps")
            for wi in range(8):
                nc.tensor.matmul(warm_ps, warm[:, 0:P], warm,
                                 start=(wi == 0), stop=(wi == 7))

            # ---- basis production: T_1..T_8 per 128-row chunk of i ----
            basis = {}

            for ic in range(IC):
                # ic == 0 runs every op on two half-tiles: the PE is already
                # warm when the kernel starts consuming, and half-granularity
                # lets the b<4 matmuls of each K-chunk start one half-op
                # earlier, which keeps the warm PE gapless during ramp-up.
                slices = ([slice(0, B_LOCAL // 2), slice(B_LOCAL // 2, B_LOCAL)]
                          if ic == 0 else [slice(0, B_LOCAL)])

                # xt on the HWDGE (sync) queue: issues in parallel with the
                # gpsimd wt stream and has lower first-byte latency.
                xt_t = xpool.tile([P, B_LOCAL], f32, tag="xt", name=f"xt_{ic}")
                for sl in slices:
                    nc.sync.dma_start(out=xt_t[:, sl],
                                      in_=xt_d[ic * P:(ic + 1) * P, sl])

                def btile(d):
                    bt = bpool.tile([P, B_LOCAL], bf16, tag=f"b_{ic}_{d}",
                                    name=f"b_{ic}_{d}")
                    basis[(ic, d)] = bt
                    return bt

                # T1 = tanh(x) (no clip: the recurrence is stable for |t|<=1
                # and T_d(+-1) is finite; deviation from the reference's
                # clip at 0.999 is ~1e-6 on y)
                t = fpool.tile([P, B_LOCAL], f32, tag="T1", name=f"t_{ic}")
                s2 = fpool.tile([P, B_LOCAL], f32, tag="sq", name=f"s2_{ic}")
                T2 = fpool.tile([P, B_LOCAL], f32, tag="T2", name=f"T2_{ic}",
                                bufs=1)
                V3 = fpool.tile([P, B_LOCAL], f32, tag="u", name=f"V3_{ic}")
                T3 = fpool.tile([P, B_LOCAL], f32, tag="T3", name=f"T3_{ic}",
                                bufs=1)
                s4 = fpool.tile([P, B_LOCAL], f32, tag="sq", name=f"s4_{ic}")
                T4 = fpool.tile([P, B_LOCAL], f32, tag="T4", name=f"T4_{ic}",
                                bufs=1)
                u5 = fpool.tile([P, B_LOCAL], bf16, tag="ub", name=f"u5_{ic}")
                s6 = fpool.tile([P, B_LOCAL], f32, tag="sq", name=f"s6_{ic}")
                u7 = fpool.tile([P, B_LOCAL], bf16, tag="ub", name=f"u7_{ic}")
                s8 = fpool.tile([P, B_LOCAL], f32, tag="sq", name=f"s8_{ic}")
                b1, b2, b3, b4 = btile(1), btile(2), btile(3), btile(4)
                b5, b6, b7, b8 = btile(5), btile(6), btile(7), btile(8)

                for sl in slices:
                    nc.scalar.activation(t[:, sl], xt_t[:, sl], AF.Tanh)
                    # DVE cast: shortens the tanh -> first-matmul chain
                    nc.vector.tensor_copy(b1[:, sl], t[:, sl])

                    # T2 = 2 t^2 - 1
                    nc.scalar.square(s2[:, sl], t[:, sl])
                    nc.vector.tensor_scalar(T2[:, sl], s2[:, sl], 2.0, -1.0,
                                            ALU.mult, ALU.add)
                    nc.scalar.copy(b2[:, sl], T2[:, sl])

                    # T3 = 2 t T2 - t = t * (2 T2 - 1)
                    nc.vector.tensor_scalar(V3[:, sl], T2[:, sl], 2.0, -1.0,
                                            ALU.mult, ALU.add)
                    nc.vector.tensor_mul(T3[:, sl], t[:, sl], V3[:, sl])
                    nc.scalar.copy(b3[:, sl], T3[:, sl])

                    # T4 = 2 T2^2 - 1
                    nc.scalar.square(s4[:, sl], T2[:, sl])
                    nc.vector.tensor_scalar(T4[:, sl], s4[:, sl], 2.0, -1.0,
                                            ALU.mult, ALU.add)
                    nc.scalar.copy(b4[:, sl], T4[:, sl])

                    # Degrees 5..8 are leaves (no downstream consumer), so
                    # they can be produced in cheaper precision/modes:
                    #   T5 = 2 T2 T3 - T1, T7 = 2 T3 T4 - T1 from bf16
                    #   operands (bf16 DVE ops run in 2x mode)
                    #   T6 = 2 T3^2 - 1, T8 = 2 T4^2 - 1 as one tensor_scalar
                    #   with direct bf16 output (fp32 squares from ACT)
                    nc.vector.tensor_mul(u5[:, sl], b2[:, sl], b3[:, sl])
                    nc.vector.scalar_tensor_tensor(b5[:, sl], u5[:, sl], 2.0,
                                                   b1[:, sl],
                                                   ALU.mult, ALU.subtract)

                    nc.scalar.square(s6[:, sl], T3[:, sl])
                    nc.vector.tensor_scalar(b6[:, sl], s6[:, sl], 2.0, -1.0,
                                            ALU.mult, ALU.add)

                    nc.vector.tensor_mul(u7[:, sl], b3[:, sl], b4[:, sl])
                    nc.vector.scalar_tensor_tensor(b7[:, sl], u7[:, sl], 2.0,
                                                   b1[:, sl],
                                                   ALU.mult, ALU.subtract)

                    nc.scalar.square(s8[:, sl], T4[:, sl])
                    nc.vector.tensor_scalar(b8[:, sl], s8[:, sl], 2.0, -1.0,
                                            ALU.mult, ALU.add)

            # bias is only consumed at the end of each o-half pass; load it
            # late so it doesn't delay the xt/wt streams.
            bias_t = cpool.tile([P, O_DIM], f32, name="bias_t")
            nc.sync.dma_start(out=bias_t, in_=bias_d[:, :])

            # ---- contraction: two o-half passes over all 64 K-chunks ----
            psums = [ppool.tile([P, ON], f32, tag=f"ps{b}", name=f"ps{b}")
                     for b in range(B_LOCAL // P)]
            # pass 0: o-half 0, all 8 batch banks (overlaps basis production)
            # pass 1a/1b: o-half 1 split in two bank halves, so the first
            # half's bias-adds + stores overlap the second half's matmuls
            # and the final tail only drains 4 banks.
            passes = [(0, 0, 8), (1, 0, 4), (1, 4, 8)]
            for pi, (oh, blo, bhi) in enumerate(passes):
                for k in range(NK):
                    ic, dm1 = divmod(k, DEG)
                    wt = wpool.tile([P, ON], bf16, tag="wt",
                                    name=f"wt_{pi}_{k}")
                    nc.gpsimd.dma_start(out=wt, in_=w_d[oh, k])
                    bt = basis[(ic, dm1 + 1)]
                    for b in range(blo, bhi):
                        nc.tensor.matmul(
                            psums[b],
                            bt[:, b * P:(b + 1) * P],
                            wt,
                            start=(k == 0),
                            stop=(k == NK - 1),
                        )
                for b in range(blo, bhi):
                    ot = opool.tile([P, ON], f32, tag="ot", name=f"ot_{pi}_{b}")
                    bias_sl = bias_t[:, oh * ON:(oh + 1) * ON]
                    if pi == 0 and b < 4:
                        # banks 0-3 gate pass 1a: drain them via ACT copy so
                        # the start=True matmuls aren't stuck behind the
                        # serial DVE bias-add chain; add bias in place later
                        # (overlaps the next pass).
                        nc.scalar.copy(ot, psums[b])
                        nc.vector.tensor_add(ot, ot, bias_sl)
                    else:
                        nc.vector.tensor_add(ot, psums[b], bias_sl)
                    nc.sync.dma_start(
                        out=y_d[b * P:(b + 1) * P, oh * ON:(oh + 1) * ON],
                        in_=ot)
    nc.compile()  # bacc legalization: splits multi-sem waits (TRN2 allows 1)
    return nc


def _get_nc():
    global _nc
    if _nc is None:
        _nc = _build_nc()
    return _nc


def _prep_inputs(x, cheby_coeffs):
    x = np.asarray(x, dtype=np.float32)
    C = np.asarray(cheby_coeffs, dtype=np.float32)
    bf16 = ml_dtypes.bfloat16

    # W[oh, k=(ic,d), p, on] = C[ic*128+p, oh*512+on, d+1]
    Wd = C[:, :, 1:]                                   # [I, O, 8]
    Wd = Wd.reshape(IC, P, OH, ON, DEG)                # [ic, p, oh, on, d]
    Wd = np.transpose(Wd, (2, 0, 4, 1, 3))             # [oh, ic, d, p, on]
    Wd = np.ascontiguousarray(Wd.reshape(OH, NK, P, ON)).astype(bf16)

    bias = C[:, :, 0].sum(axis=0, dtype=np.float64).astype(np.float32)
    bias_rep = np.ascontiguousarray(np.broadcast_to(bias, (P, O_DIM)))

    in_maps = []
    for c in range(N_CORES):
        xt = np.ascontiguousarray(x[c * B_LOCAL:(c + 1) * B_LOCAL, :].T)
        in_maps.append({"xt": xt, "w": Wd, "bias": bias_rep})
    return in_maps


def kernel(x, cheby_coeffs):
    global last_results
    nc = _get_nc()
    in_maps = _prep_inputs(x, cheby_coeffs)
    last_results = run_bass_kernel_spmd(nc, in_maps,
                                        core_ids=list(range(N_CORES)))
    y = np.concatenate([r["y"] for r in last_results.results], axis=0)
    return y
